# revision 28
# baseline (speedup 1.0000x reference)
"""Causal single-head attention (B=4, T=4096, C=1024, H=64) on 8 trn2 cores.

Sharding (v4, split-K): 2 cores per batch element. Core parity h takes the
GLOBAL key tiles {4g+h, 4g+2+h : g in 0..7} (every other 128-key tile) for
ALL 8 query blocks of 512, computing flash-style partial numerators and
denominators that the host combines (A+B, then divide). Both cores run one
identical program on 36 (query-block, key-tile-pair) items -- perfectly
balanced causal work with zero padding.

Host xk layout per core: for g in 0..7, 512 columns = global key tiles
[4g+h, 4g+2+h, 4g+1-h, 4g+3-h] (own diag tiles first). Block g's queries
are xk cols [512g, 512(g+1)) (host unpermutes the output); its keys are
"positions" [0, 2(g+1)) = the first 2 column-blocks of groups 0..g.

v12 schedule (vs v3; 95.8us -> 83.9us measured):
  - one sync-queue HBM stream in consumption order: consts, then x in
    512KB ch-pair chunks (256KB chunks run ~210GB/s -- 16KB/SDMA engine
    doesn't amortize; 512KB reaches ~300+GB/s)
  - HAM clock management: warmup matmuls + dummy matmuls between the
    DMA-gated gp0 units and between phase-3 items keep PE duty above the
    idle-throttle threshold, so the whole kernel runs at 2.4GHz (the
    ACT-bound steady state alone is ~57% PE duty, which re-throttles)
  - gp0 projects Q first (its 4-copy fin is the longest pole to the first
    scores; the qp0 duplicate copies run on ACT, idle before the first exp)
  - engine queues execute in order: proj units for group-pair p+1 are
    dripped BETWEEN phase p's items, BEFORE each item (O-flushes and
    scores carry ACT-chain waits; anything behind them inherits the gate)
  - blocks ascending, item order [g, 0..g-1] (masked diag first); global
    depth-2 flush FIFO software-pipelines block boundaries; O is a single
    full-contraction matmul per position into one po accumulator
  - out DMAs ride the sync queue behind the xk loads; sa staging tiles
    stay live in SBUF (bufs=8) so nothing stalls on them
"""

import numpy as np
import ml_dtypes

import concourse.bass as bass
import concourse.bacc as bacc
import concourse.tile as tile
from concourse import mybir
from concourse.bass_utils import run_bass_kernel_spmd

B, T, C, H = 4, 4096, 1024, 64
N_CORES = 8
NCH = C // 128       # 8 contraction chunks
NG = 8               # query blocks (512 each) per batch
NPOS = 16            # key tile positions per core
F32 = mybir.dt.float32
BF16 = mybir.dt.bfloat16

_nc_cache = {}


def build_module():
    if "nc" in _nc_cache:
        return _nc_cache["nc"]
    nc = bacc.Bacc("TRN2", target_bir_lowering=False, debug=False,
                   num_devices=N_CORES)
    xk = nc.dram_tensor("xk", [C, T], BF16, kind="ExternalInput").ap()
    wq = nc.dram_tensor("wq", [128, NCH * H], BF16, kind="ExternalInput").ap()
    wk = nc.dram_tensor("wk", [128, NCH * H], BF16, kind="ExternalInput").ap()
    wv = nc.dram_tensor("wv", [128, NCH * H], BF16, kind="ExternalInput").ap()
    ident2 = nc.dram_tensor("ident2", [128, 64], BF16,
                            kind="ExternalInput").ap()
    maskd = nc.dram_tensor("maskd", [128, 1024], BF16,
                           kind="ExternalInput").ap()
    # per-block partials: rows 0:64 = O' numerator^T, row 64 = denominator
    out = nc.dram_tensor("out", [NG, H + 1, 512], F32,
                         kind="ExternalOutput").ap()

    with tile.TileContext(nc) as tc:
        with (
            tc.tile_pool(name="consts", bufs=1) as consts,
            tc.tile_pool(name="vtmp", bufs=2) as vtmp_pool,
            tc.tile_pool(name="exps", bufs=8) as exps_pool,
            tc.tile_pool(name="fin", bufs=8) as fin_pool,
            tc.tile_pool(name="ps_s", bufs=2, space="PSUM") as ps_s,
            tc.tile_pool(name="ps_o", bufs=2, space="PSUM") as ps_o,
            tc.tile_pool(name="ps_p", bufs=2, space="PSUM") as ps_p,
        ):
            xk_r = xk.rearrange("(ch p) t -> p ch t", p=128)

            # ---- everything rides the sync HWDGE queue in consumption
            # order: wk first (the K units are the first consumers), then
            # group-pair 0 of x, then the remaining consts (needed only once
            # V/Q units run, ~6us later), then group-pairs 1-3. One queue =
            # one full-rate HBM stream with no semaphore-lane collisions.
            w_sb = {}
            for name in ("wq", "wk", "wv"):
                w_sb[name] = consts.tile([128, NCH * H], BF16,
                                         name=f"{name}_sb")
            id2_sb = consts.tile([128, 64], BF16, name="id2_sb")
            mask_sb = consts.tile([128, 1024], BF16, name="mask_sb")
            xt = consts.tile([128, NCH, NG, 4, 128], BF16, name="xt")

            def xt_dmas(gp, nch_per_dma):
                for c0 in range(0, NCH, nch_per_dma):
                    nc.sync.dma_start(
                        xt[:, c0:c0 + nch_per_dma, 2 * gp:2 * gp + 2, :, :],
                        xk_r[:, c0:c0 + nch_per_dma,
                             1024 * gp:1024 * (gp + 1)])

            # 512KB ch-pair chunks throughout: small chunks run well under
            # the 358GB/s HBM rate (16KB per SDMA engine doesn't amortize),
            # so fewer/bigger beats per-ch latency. All consts lead the
            # stream so every unit kind (incl. V transposes on id2) can
            # chase gp0's chunks without gating.
            # only wq must precede gp0's chunks (units are Q-first); the
            # other consts ride between gp0 and gp1, well before their
            # first consumers (K/V units, V transposes, first diag mask)
            nc.sync.dma_start(w_sb["wq"][:], wq)
            xt_dmas(0, 2)
            nc.sync.dma_start(w_sb["wk"][:], wk)
            nc.sync.dma_start(w_sb["wv"][:], wv)
            nc.sync.dma_start(id2_sb[:], ident2)
            nc.sync.dma_start(mask_sb[:], maskd)
            for gp in range(1, 4):
                xt_dmas(gp, 2)

            # ---- PE clock warmup: dense matmuls covering the gap between
            # the engine preamble and the first xk chunk landing.
            wmt = consts.tile([128, 128], BF16, name="wmt")
            wmt2 = consts.tile([128, 512], BF16, name="wmt2")
            nc.vector.memset(wmt[:], 0.0)
            nc.vector.memset(wmt2[:], 0.0)
            wps = ps_s.tile([128, 512], F32, tag="ps", name="wm")
            NWARM = 10
            for wi in range(NWARM):
                nc.tensor.matmul(wps[:], wmt[:], wmt2[:], start=(wi == 0),
                                 stop=(wi == NWARM - 1),
                                 skip_group_check=True)

            def pe_dummy(n, tag):
                # keeps PE activity above the HAM idle threshold during
                # DMA-gated or ACT-bound stretches ("ps" during the initial
                # chase before any scores exist, "pp" during phase 3 when
                # the projection pool is idle)
                wd = ps_p.tile([128, 512], F32, tag=tag, name=f"wd{n}") \
                    if tag == "pp" else \
                    ps_s.tile([128, 512], F32, tag=tag, name=f"wd{n}")
                nc.tensor.matmul(wd[:], wmt[:], wmt2[:], start=True,
                                 stop=True, skip_group_check=True)

            # ---- persistent activations ----
            # kt2x: position 2i+par at [64*par:(par+1)*64, 128i:128(i+1)]
            kt2x = consts.tile([128, 8 * 128], BF16, name="kt2x")
            # qt2x: Q^T in xk column order, duplicated on partition halves
            qt2x = consts.tile([128, T], BF16, name="qt2x")
            v_all = consts.tile([128, NPOS, H + 1], BF16, name="v_all")
            nc.vector.memset(v_all[:, :, H], 1.0)

            inv_sqrt_h = 1.0 / np.sqrt(np.float32(H))

            def wslice(wname, ch):
                return w_sb[wname][:, ch * H:(ch + 1) * H]

            # ---------- projection units (generators of emission thunks) ---
            def gen_projQ(qp):
                """Q over groups (2qp, 2qp+1) -> qt2x cols [1024qp,+1024)."""
                pq = ps_p.tile([128, 512], F32, tag="pp", name=f"pq{qp}")

                def unit(ch):
                    nc.tensor.matmul(pq[0:64, :], wslice("wq", ch),
                                     xt[:, ch, 2 * qp, :, :],
                                     start=(ch == 0), stop=(ch == NCH - 1))
                    nc.tensor.matmul(pq[64:128, :], wslice("wq", ch),
                                     xt[:, ch, 2 * qp + 1, :, :],
                                     start=(ch == 0), stop=(ch == NCH - 1),
                                     tile_position=(0, 64))

                def fin():
                    # write half 0:64 from psum (2 casts), then duplicate
                    # to 64:128 with one bf16 SBUF->SBUF copy (2x DVE mode,
                    # ~0.7us for the full 1024 cols) -- this fin is on the
                    # next phase's first-scores critical path. qp 0's
                    # duplicates go on ACT, idle before the first exp.
                    for half in range(2):
                        sl = pq[64 * half:64 * (half + 1), :]
                        dst = slice(1024 * qp + half * 512,
                                    1024 * qp + (half + 1) * 512)
                        nc.vector.tensor_copy(qt2x[0:64, dst], sl)
                        if qp == 0:
                            nc.scalar.copy(qt2x[64:128, dst], sl)
                    if qp > 0:
                        cs = slice(1024 * qp, 1024 * (qp + 1))
                        nc.vector.tensor_copy(qt2x[64:128, cs],
                                              qt2x[0:64, cs])

                return ([(lambda ch=ch: unit(ch)) for ch in range(NCH)]
                        + [fin])

            def gen_projK(ku):
                """K for positions (4ku..4ku+3) = groups (2ku, 2ku+1)."""
                pk = ps_p.tile([128, 256], F32, tag="pp", name=f"pk{ku}")

                def unit(ch):
                    nc.tensor.matmul(pk[0:64, :], wslice("wk", ch),
                                     xt[:, ch, 2 * ku:2 * ku + 2, 0, :],
                                     start=(ch == 0), stop=(ch == NCH - 1))
                    nc.tensor.matmul(pk[64:128, :], wslice("wk", ch),
                                     xt[:, ch, 2 * ku:2 * ku + 2, 1, :],
                                     start=(ch == 0), stop=(ch == NCH - 1),
                                     tile_position=(0, 64))

                def fin():
                    cs = slice(256 * ku, 256 * (ku + 1))
                    nc.vector.tensor_copy(kt2x[0:64, cs], pk[0:64, :])
                    nc.vector.tensor_copy(kt2x[64:128, cs], pk[64:128, :])

                return [(lambda ch=ch: unit(ch)) for ch in range(NCH)] + [fin]

            def gen_projV(ku):
                pv = ps_p.tile([128, 256], F32, tag="pp", name=f"pv{ku}")

                def unit(ch):
                    nc.tensor.matmul(pv[0:64, :], wslice("wv", ch),
                                     xt[:, ch, 2 * ku:2 * ku + 2, 0, :],
                                     start=(ch == 0), stop=(ch == NCH - 1))
                    nc.tensor.matmul(pv[64:128, :], wslice("wv", ch),
                                     xt[:, ch, 2 * ku:2 * ku + 2, 1, :],
                                     start=(ch == 0), stop=(ch == NCH - 1),
                                     tile_position=(0, 64))

                def fin():
                    vt = vtmp_pool.tile([128, 256], BF16, tag="vt",
                                        name=f"vt{ku}")
                    nc.vector.tensor_copy(vt[:], pv[:])
                    # half0 = sub 0 of groups (2ku, 2ku+1) = positions
                    # (4ku, 4ku+2); half1 = sub 1 = (4ku+1, 4ku+3)
                    for half in range(2):
                        for t in range(2):
                            p = 4 * ku + 2 * t + half
                            ptr = ps_p.tile([128, 64], BF16, tag="pp",
                                            name=f"ptr{p}")
                            nc.tensor.transpose(
                                ptr[:],
                                vt[64 * half:64 * (half + 1),
                                   t * 128:(t + 1) * 128],
                                id2_sb[64 * half:64 * (half + 1), :])
                            nc.vector.tensor_copy(v_all[:, p, 0:H], ptr[:])

                return [(lambda ch=ch: unit(ch)) for ch in range(NCH)] + [fin]

            # ---------- attention ----------
            # Global flush FIFO (depth 2): O matmuls trail their exps by two
            # items so the diag mask is off the in-order PE queue's critical
            # path, and block boundaries software-pipeline (block g's last
            # flushes interleave with block g+1's first scores/exps).
            attn_state = {}
            pending = []

            def attn_begin(g):
                po = ps_o.tile([H + 1, 512], F32, tag="po", name=f"po{g}")
                attn_state[g] = dict(po=po, nfl=0)

            def attn_finalize(g):
                # stage psum to SBUF (sa tiles stay live, bufs=8) and queue
                # the out DMA on sync behind the xk loads
                sa = fin_pool.tile([H + 1, 512], F32, tag="sa", name=f"sa{g}")
                nc.vector.tensor_copy(sa[:], attn_state[g]["po"][:])
                nc.sync.dma_start(out[g], sa[:])

            def attn_flush_one():
                g, i, es2 = pending.pop(0)
                st = attn_state[g]
                n = st["nfl"]
                st["nfl"] += 1
                for t in range(2):
                    p = 2 * i + t
                    cs = slice(t * 512, (t + 1) * 512)
                    nc.tensor.matmul(
                        st["po"][:], v_all[:, p, :], es2[:, cs],
                        start=(n == 0 and t == 0), stop=(n == g and t == 1),
                        skip_group_check=True)
                if st["nfl"] == g + 1:
                    attn_finalize(g)

            def attn_item(g, i):
                qs_a = qt2x[0:64, g * 512:(g + 1) * 512]
                qs_b = qt2x[64:128, g * 512:(g + 1) * 512]
                ps = ps_s.tile([128, 1024], F32, tag="ps", name=f"s{g}_{i}")
                nc.tensor.matmul(ps[:, 0:512],
                                 kt2x[0:64, 128 * i:128 * (i + 1)],
                                 qs_a, start=True, stop=True)
                nc.tensor.matmul(ps[:, 512:1024],
                                 kt2x[64:128, 128 * i:128 * (i + 1)],
                                 qs_b, start=True, stop=True,
                                 tile_position=(64, 0))
                es2 = exps_pool.tile([128, 1024], BF16, tag="es",
                                     name=f"e{g}_{i}")
                nc.scalar.activation(es2[:], ps[:],
                                     mybir.ActivationFunctionType.Exp,
                                     scale=float(inv_sqrt_h))
                if i == g:
                    # diagonal pair: zero the causally-invalid entries
                    nc.vector.tensor_tensor(
                        es2[:], es2[:], mask_sb[:], op=mybir.AluOpType.mult)
                pending.append((g, i, es2))
                while len(pending) > 2:
                    attn_flush_one()

            # ---------- emission ----------
            def gp_units(gp):
                # Q first, always: the next phase's first scores need the
                # Q and K fins of this group-pair, while V feeds only the
                # O-flushes that trail by two items. Emitting Q last (K,V,Q)
                # was costing ~4-5us per phase boundary: Q's units+fin sat
                # at the back of the in-order PE/DVE queues.
                return (gen_projQ(gp) + gen_projK(gp) + gen_projV(gp))

            # gp0 runs solid, chasing the DMA stream; dummies between the
            # DMA-gated units keep PE duty above the HAM warm threshold so
            # the projections run at 2.4GHz
            ndum = [0]

            def chase_dummy():
                pe_dummy(ndum[0], "ps")
                ndum[0] += 1

            units0 = gp_units(0)
            for j, f in enumerate(units0):
                f()
                if j < 8:           # only the DMA-paced K pass needs fill;
                    chase_dummy()   # ~1.3us gap per 512KB chunk arrival
                    chase_dummy()
                    chase_dummy()

            # Drip-feed each phase's NEXT group-pair projections evenly
            # between its attention items. The O-flush matmuls wait on
            # exps/masks, so proj units emitted between items run during
            # the ACT drain; emitting them after the phase (v7/v8) leaves
            # them stuck in the in-order PE queue behind O-flushes, which
            # serializes every phase boundary by the whole exp drain.
            for ph in range(4):
                nxt = gp_units(ph + 1) if ph < 3 else []
                items = []
                for g in (2 * ph, 2 * ph + 1):
                    items.append(("begin", g))
                    for i in [g] + list(range(g)):
                        items.append(("item", g, i))
                n_items = sum(1 for it in items if it[0] == "item")
                j = 0
                k = 0
                for it in items:
                    if it[0] == "begin":
                        attn_begin(it[1])
                        continue
                    # drip BEFORE the item: the item's scores wait on fins,
                    # and anything emitted after them in the in-order PE
                    # queue would inherit that gate even when its own DMA
                    # data landed long ago
                    j += 1
                    target = len(nxt) * j // n_items
                    while k < target:
                        nxt[k]()
                        k += 1
                    attn_item(it[1], it[2])
                    if ph == 3:
                        # no projections left to drip: keep PE activity
                        # above the HAM idle threshold
                        pe_dummy(ndum[0], "pp")
                        ndum[0] += 1
            while pending:
                attn_flush_one()
    nc.compile()
    _nc_cache["nc"] = nc
    return nc


def _sub_order(h):
    return [h, 2 + h, 1 - h, 3 - h]


def _core_inputs(x, Wq, Wk, Wv, core):
    b, h = core // 2, core % 2
    sub = _sub_order(h)
    xkm = np.empty((C, T), dtype=np.float32)
    xb = np.asarray(x[b], dtype=np.float32)  # [T, C]
    for g in range(NG):
        for a, s in enumerate(sub):
            tlo = 128 * (4 * g + s)
            xkm[:, 512 * g + 128 * a: 512 * g + 128 * (a + 1)] = \
                xb[tlo:tlo + 128, :].T
    id2 = np.zeros((128, 64), dtype=np.float32)
    id2[:64] = np.eye(64, dtype=np.float32)
    id2[64:] = np.eye(64, dtype=np.float32)
    # diagonal-pair mask: cols [0,512) vs own tile s=h; [512,1024) vs s=2+h
    k = np.arange(128)[:, None]
    qcol = np.arange(512)[None, :]
    qoff = 128 * np.array(sub)[qcol // 128] + qcol % 128
    m0 = (qoff >= 128 * h + k)
    m1 = (qoff >= 128 * (2 + h) + k)
    mask = np.concatenate([m0, m1], axis=1).astype(np.float32)
    bf = ml_dtypes.bfloat16

    def warr(W):
        w = np.asarray(W, dtype=np.float32)
        return np.ascontiguousarray(
            w.reshape(NCH, 128, H).transpose(1, 0, 2).reshape(128, NCH * H)
            .astype(bf))

    return {
        "xk": np.ascontiguousarray(xkm.astype(bf)),
        "wq": warr(Wq),
        "wk": warr(Wk),
        "wv": warr(Wv),
        "ident2": id2.astype(bf),
        "maskd": np.ascontiguousarray(mask.astype(bf)),
    }


def kernel(x, Wq, Wk, Wv):
    x = np.asarray(x, dtype=np.float32)
    nc = build_module()
    in_maps = [_core_inputs(x, Wq, Wk, Wv, c) for c in range(N_CORES)]
    res = run_bass_kernel_spmd(nc, in_maps, core_ids=list(range(N_CORES)))
    out = np.empty((B, T, H), dtype=np.float32)
    inv = [np.argsort(_sub_order(h)) for h in range(2)]
    for b in range(B):
        pa = res.results[2 * b]["out"].astype(np.float64)   # [8, 65, 512]
        pb = res.results[2 * b + 1]["out"].astype(np.float64)
        # unpermute each core's query columns to global order, then combine
        pa = pa.reshape(NG, H + 1, 4, 128)[:, :, inv[0], :]
        pb = pb.reshape(NG, H + 1, 4, 128)[:, :, inv[1], :]
        num = pa[:, :H] + pb[:, :H]                  # [8, 64, 4, 128]
        den = pa[:, H] + pb[:, H]                    # [8, 4, 128]
        o = num / den[:, None, :, :]                 # [8, 64, 4, 128]
        out[b] = (o.transpose(0, 2, 3, 1)            # [8, 4, 128, 64]
                  .reshape(T, H).astype(np.float32))
    return out


# revision 31
# speedup vs baseline: 1.1590x; 1.1590x over previous
"""Causal single-head attention (B=4, T=4096, C=1024, H=64) on 8 trn2 cores.

Sharding (v4, split-K): 2 cores per batch element. Core parity h takes the
GLOBAL key tiles {4g+h, 4g+2+h : g in 0..7} (every other 128-key tile) for
ALL 8 query blocks of 512, computing flash-style partial numerators and
denominators that the host combines (A+B, then divide). Both cores run one
identical program on 36 (query-block, key-tile-pair) items -- perfectly
balanced causal work with zero padding.

Host xk layout per core: for g in 0..7, 512 columns = global key tiles
[4g+h, 4g+2+h, 4g+1-h, 4g+3-h] (own diag tiles first). Block g's queries
are xk cols [512g, 512(g+1)) (host unpermutes the output); its keys are
"positions" [0, 2(g+1)) = the first 2 column-blocks of groups 0..g.

v13 schedule (vs v3; 95.8us -> 83.1us measured):
  - one sync-queue HBM stream in consumption order: consts, then x in
    512KB ch-pair chunks (256KB chunks run ~210GB/s -- 16KB/SDMA engine
    doesn't amortize; 512KB reaches ~300+GB/s)
  - HAM clock management: warmup matmuls + dummy matmuls between the
    DMA-gated gp0 units and between phase-3 items keep PE duty above the
    idle-throttle threshold, so the whole kernel runs at 2.4GHz (the
    ACT-bound steady state alone is ~57% PE duty, which re-throttles)
  - every group-pair projects Q first (the next phase's first scores need
    the Q and K fins; V feeds only O-flushes trailing by two items; the
    qp0 duplicate copies run on ACT, idle before the first exp)
  - engine queues execute in order: proj units for group-pair p+1 are
    dripped BETWEEN phase p's items, BEFORE each item (O-flushes and
    scores carry ACT-chain waits; anything behind them inherits the gate)
  - blocks ascending, item order [g, 0..g-1] (masked diag first); global
    depth-2 flush FIFO software-pipelines block boundaries; O is a single
    full-contraction matmul per position into one po accumulator
  - out DMAs ride the sync queue behind the xk loads; sa staging tiles
    stay live in SBUF (bufs=8) so nothing stalls on them
"""

import numpy as np
import ml_dtypes

import concourse.bass as bass
import concourse.bacc as bacc
import concourse.tile as tile
from concourse import mybir
from concourse.bass_utils import run_bass_kernel_spmd

B, T, C, H = 4, 4096, 1024, 64
N_CORES = 8
NCH = C // 128       # 8 contraction chunks
NG = 8               # query blocks (512 each) per batch
NPOS = 16            # key tile positions per core
F32 = mybir.dt.float32
BF16 = mybir.dt.bfloat16

_nc_cache = {}


def build_module():
    if "nc" in _nc_cache:
        return _nc_cache["nc"]
    nc = bacc.Bacc("TRN2", target_bir_lowering=False, debug=False,
                   num_devices=N_CORES)
    xk = nc.dram_tensor("xk", [C, T], BF16, kind="ExternalInput").ap()
    wq = nc.dram_tensor("wq", [128, NCH * H], BF16, kind="ExternalInput").ap()
    wk = nc.dram_tensor("wk", [128, NCH * H], BF16, kind="ExternalInput").ap()
    wv = nc.dram_tensor("wv", [128, NCH * H], BF16, kind="ExternalInput").ap()
    ident2 = nc.dram_tensor("ident2", [128, 64], BF16,
                            kind="ExternalInput").ap()
    maskd = nc.dram_tensor("maskd", [128, 1024], BF16,
                           kind="ExternalInput").ap()
    # per-block partials: rows 0:64 = O' numerator^T, row 64 = denominator
    out = nc.dram_tensor("out", [NG, H + 1, 512], F32,
                         kind="ExternalOutput").ap()

    with tile.TileContext(nc) as tc:
        with (
            tc.tile_pool(name="consts", bufs=1) as consts,
            tc.tile_pool(name="vtmp", bufs=2) as vtmp_pool,
            tc.tile_pool(name="exps", bufs=8) as exps_pool,
            tc.tile_pool(name="fin", bufs=8) as fin_pool,
            tc.tile_pool(name="ps_s", bufs=2, space="PSUM") as ps_s,
            tc.tile_pool(name="ps_o", bufs=2, space="PSUM") as ps_o,
            tc.tile_pool(name="ps_p", bufs=2, space="PSUM") as ps_p,
        ):
            xk_r = xk.rearrange("(ch p) t -> p ch t", p=128)

            # ---- everything rides the sync HWDGE queue in consumption
            # order: wk first (the K units are the first consumers), then
            # group-pair 0 of x, then the remaining consts (needed only once
            # V/Q units run, ~6us later), then group-pairs 1-3. One queue =
            # one full-rate HBM stream with no semaphore-lane collisions.
            w_sb = {}
            for name in ("wq", "wk", "wv"):
                w_sb[name] = consts.tile([128, NCH * H], BF16,
                                         name=f"{name}_sb")
            id2_sb = consts.tile([128, 64], BF16, name="id2_sb")
            mask_sb = consts.tile([128, 1024], BF16, name="mask_sb")
            xt = consts.tile([128, NCH, NG, 4, 128], BF16, name="xt")

            def xt_dmas(gp, nch_per_dma):
                for c0 in range(0, NCH, nch_per_dma):
                    nc.sync.dma_start(
                        xt[:, c0:c0 + nch_per_dma, 2 * gp:2 * gp + 2, :, :],
                        xk_r[:, c0:c0 + nch_per_dma,
                             1024 * gp:1024 * (gp + 1)])

            # 512KB ch-pair chunks throughout: small chunks run well under
            # the 358GB/s HBM rate (16KB per SDMA engine doesn't amortize),
            # so fewer/bigger beats per-ch latency. All consts lead the
            # stream so every unit kind (incl. V transposes on id2) can
            # chase gp0's chunks without gating.
            nc.sync.dma_start(w_sb["wk"][:], wk)
            nc.sync.dma_start(w_sb["wv"][:], wv)
            nc.sync.dma_start(w_sb["wq"][:], wq)
            nc.sync.dma_start(id2_sb[:], ident2)
            nc.sync.dma_start(mask_sb[:], maskd)
            for gp in range(4):
                xt_dmas(gp, 2)

            # ---- PE clock warmup: dense matmuls covering the gap between
            # the engine preamble and the first xk chunk landing.
            wmt = consts.tile([128, 128], BF16, name="wmt")
            wmt2 = consts.tile([128, 512], BF16, name="wmt2")
            nc.vector.memset(wmt[:], 0.0)
            nc.vector.memset(wmt2[:], 0.0)
            wps = ps_s.tile([128, 512], F32, tag="ps", name="wm")
            NWARM = 10
            for wi in range(NWARM):
                nc.tensor.matmul(wps[:], wmt[:], wmt2[:], start=(wi == 0),
                                 stop=(wi == NWARM - 1),
                                 skip_group_check=True)

            def pe_dummy(n, tag):
                # keeps PE activity above the HAM idle threshold during
                # DMA-gated or ACT-bound stretches ("ps" during the initial
                # chase before any scores exist, "pp" during phase 3 when
                # the projection pool is idle)
                wd = ps_p.tile([128, 512], F32, tag=tag, name=f"wd{n}") \
                    if tag == "pp" else \
                    ps_s.tile([128, 512], F32, tag=tag, name=f"wd{n}")
                nc.tensor.matmul(wd[:], wmt[:], wmt2[:], start=True,
                                 stop=True, skip_group_check=True)

            # ---- persistent activations ----
            # kt2x: position 2i+par at [64*par:(par+1)*64, 128i:128(i+1)]
            kt2x = consts.tile([128, 8 * 128], BF16, name="kt2x")
            # qt2x: Q^T in xk column order, duplicated on partition halves
            qt2x = consts.tile([128, T], BF16, name="qt2x")
            v_all = consts.tile([128, NPOS, H + 1], BF16, name="v_all")
            nc.vector.memset(v_all[:, :, H], 1.0)

            inv_sqrt_h = 1.0 / np.sqrt(np.float32(H))

            def wslice(wname, ch):
                return w_sb[wname][:, ch * H:(ch + 1) * H]

            # ---------- projection units (generators of emission thunks) ---
            def gen_projQ(qp):
                """Q over groups (2qp, 2qp+1) -> qt2x cols [1024qp,+1024)."""
                pq = ps_p.tile([128, 512], F32, tag="pp", name=f"pq{qp}")

                def unit(ch):
                    nc.tensor.matmul(pq[0:64, :], wslice("wq", ch),
                                     xt[:, ch, 2 * qp, :, :],
                                     start=(ch == 0), stop=(ch == NCH - 1))
                    nc.tensor.matmul(pq[64:128, :], wslice("wq", ch),
                                     xt[:, ch, 2 * qp + 1, :, :],
                                     start=(ch == 0), stop=(ch == NCH - 1),
                                     tile_position=(0, 64))

                def fin():
                    # write both partition halves (T8 score tiles read the
                    # 64:128 duplicate). qp 0 is on the first-scores
                    # critical path: its duplicate copies go on ACT, which
                    # is idle before the first exp, halving the path.
                    for half in range(2):
                        sl = pq[64 * half:64 * (half + 1), :]
                        dst = slice(1024 * qp + half * 512,
                                    1024 * qp + (half + 1) * 512)
                        nc.vector.tensor_copy(qt2x[0:64, dst], sl)
                        if qp == 0:
                            nc.scalar.copy(qt2x[64:128, dst], sl)
                        else:
                            nc.vector.tensor_copy(qt2x[64:128, dst], sl)

                return ([(lambda ch=ch: unit(ch)) for ch in range(NCH)]
                        + [fin])

            def gen_projK(ku):
                """K for positions (4ku..4ku+3) = groups (2ku, 2ku+1)."""
                pk = ps_p.tile([128, 256], F32, tag="pp", name=f"pk{ku}")

                def unit(ch):
                    nc.tensor.matmul(pk[0:64, :], wslice("wk", ch),
                                     xt[:, ch, 2 * ku:2 * ku + 2, 0, :],
                                     start=(ch == 0), stop=(ch == NCH - 1))
                    nc.tensor.matmul(pk[64:128, :], wslice("wk", ch),
                                     xt[:, ch, 2 * ku:2 * ku + 2, 1, :],
                                     start=(ch == 0), stop=(ch == NCH - 1),
                                     tile_position=(0, 64))

                def fin():
                    cs = slice(256 * ku, 256 * (ku + 1))
                    nc.vector.tensor_copy(kt2x[0:64, cs], pk[0:64, :])
                    nc.vector.tensor_copy(kt2x[64:128, cs], pk[64:128, :])

                return [(lambda ch=ch: unit(ch)) for ch in range(NCH)] + [fin]

            def gen_projV(ku):
                pv = ps_p.tile([128, 256], F32, tag="pp", name=f"pv{ku}")

                def unit(ch):
                    nc.tensor.matmul(pv[0:64, :], wslice("wv", ch),
                                     xt[:, ch, 2 * ku:2 * ku + 2, 0, :],
                                     start=(ch == 0), stop=(ch == NCH - 1))
                    nc.tensor.matmul(pv[64:128, :], wslice("wv", ch),
                                     xt[:, ch, 2 * ku:2 * ku + 2, 1, :],
                                     start=(ch == 0), stop=(ch == NCH - 1),
                                     tile_position=(0, 64))

                def fin():
                    vt = vtmp_pool.tile([128, 256], BF16, tag="vt",
                                        name=f"vt{ku}")
                    nc.vector.tensor_copy(vt[:], pv[:])
                    # half0 = sub 0 of groups (2ku, 2ku+1) = positions
                    # (4ku, 4ku+2); half1 = sub 1 = (4ku+1, 4ku+3)
                    for half in range(2):
                        for t in range(2):
                            p = 4 * ku + 2 * t + half
                            ptr = ps_p.tile([128, 64], BF16, tag="pp",
                                            name=f"ptr{p}")
                            nc.tensor.transpose(
                                ptr[:],
                                vt[64 * half:64 * (half + 1),
                                   t * 128:(t + 1) * 128],
                                id2_sb[64 * half:64 * (half + 1), :])
                            nc.vector.tensor_copy(v_all[:, p, 0:H], ptr[:])

                return [(lambda ch=ch: unit(ch)) for ch in range(NCH)] + [fin]

            # ---------- attention ----------
            # Global flush FIFO (depth 2): O matmuls trail their exps by two
            # items so the diag mask is off the in-order PE queue's critical
            # path, and block boundaries software-pipeline (block g's last
            # flushes interleave with block g+1's first scores/exps).
            attn_state = {}
            pending = []

            def attn_begin(g):
                po = ps_o.tile([H + 1, 512], F32, tag="po", name=f"po{g}")
                attn_state[g] = dict(po=po, nfl=0)

            def attn_finalize(g):
                # stage psum to SBUF (sa tiles stay live, bufs=8) and queue
                # the out DMA on sync behind the xk loads
                sa = fin_pool.tile([H + 1, 512], F32, tag="sa", name=f"sa{g}")
                nc.vector.tensor_copy(sa[:], attn_state[g]["po"][:])
                nc.sync.dma_start(out[g], sa[:])

            def attn_flush_one():
                g, i, es2 = pending.pop(0)
                st = attn_state[g]
                n = st["nfl"]
                st["nfl"] += 1
                for t in range(2):
                    p = 2 * i + t
                    cs = slice(t * 512, (t + 1) * 512)
                    nc.tensor.matmul(
                        st["po"][:], v_all[:, p, :], es2[:, cs],
                        start=(n == 0 and t == 0), stop=(n == g and t == 1),
                        skip_group_check=True)
                if st["nfl"] == g + 1:
                    attn_finalize(g)

            def attn_item(g, i):
                qs_a = qt2x[0:64, g * 512:(g + 1) * 512]
                qs_b = qt2x[64:128, g * 512:(g + 1) * 512]
                ps = ps_s.tile([128, 1024], F32, tag="ps", name=f"s{g}_{i}")
                nc.tensor.matmul(ps[:, 0:512],
                                 kt2x[0:64, 128 * i:128 * (i + 1)],
                                 qs_a, start=True, stop=True)
                nc.tensor.matmul(ps[:, 512:1024],
                                 kt2x[64:128, 128 * i:128 * (i + 1)],
                                 qs_b, start=True, stop=True,
                                 tile_position=(64, 0))
                es2 = exps_pool.tile([128, 1024], BF16, tag="es",
                                     name=f"e{g}_{i}")
                nc.scalar.activation(es2[:], ps[:],
                                     mybir.ActivationFunctionType.Exp,
                                     scale=float(inv_sqrt_h))
                if i == g:
                    # diagonal pair: zero the causally-invalid entries
                    nc.vector.tensor_tensor(
                        es2[:], es2[:], mask_sb[:], op=mybir.AluOpType.mult)
                pending.append((g, i, es2))
                while len(pending) > 2:
                    attn_flush_one()

            # ---------- emission ----------
            def gp_units(gp):
                # Q first, always: the next phase's first scores need the
                # Q and K fins of this group-pair, while V feeds only the
                # O-flushes that trail by two items. Emitting Q last (K,V,Q)
                # was costing ~4-5us per phase boundary: Q's units+fin sat
                # at the back of the in-order PE/DVE queues.
                return (gen_projQ(gp) + gen_projK(gp) + gen_projV(gp))

            # gp0 runs solid, chasing the DMA stream; dummies between the
            # DMA-gated units keep PE duty above the HAM warm threshold so
            # the projections run at 2.4GHz
            ndum = [0]

            def chase_dummy():
                pe_dummy(ndum[0], "ps")
                ndum[0] += 1

            units0 = gp_units(0)
            for j, f in enumerate(units0):
                f()
                if j < 8:           # only the DMA-paced K pass needs fill;
                    chase_dummy()   # ~1.3us gap per 512KB chunk arrival
                    chase_dummy()
                    chase_dummy()

            # Drip-feed each phase's NEXT group-pair projections evenly
            # between its attention items. The O-flush matmuls wait on
            # exps/masks, so proj units emitted between items run during
            # the ACT drain; emitting them after the phase (v7/v8) leaves
            # them stuck in the in-order PE queue behind O-flushes, which
            # serializes every phase boundary by the whole exp drain.
            for ph in range(4):
                nxt = gp_units(ph + 1) if ph < 3 else []
                items = []
                for g in (2 * ph, 2 * ph + 1):
                    items.append(("begin", g))
                    for i in [g] + list(range(g)):
                        items.append(("item", g, i))
                n_items = sum(1 for it in items if it[0] == "item")
                j = 0
                k = 0
                for it in items:
                    if it[0] == "begin":
                        attn_begin(it[1])
                        continue
                    # drip BEFORE the item: the item's scores wait on fins,
                    # and anything emitted after them in the in-order PE
                    # queue would inherit that gate even when its own DMA
                    # data landed long ago
                    j += 1
                    target = len(nxt) * j // n_items
                    while k < target:
                        nxt[k]()
                        k += 1
                    attn_item(it[1], it[2])
                    if ph == 3:
                        # no projections left to drip: keep PE activity
                        # above the HAM idle threshold
                        pe_dummy(ndum[0], "pp")
                        ndum[0] += 1
            while pending:
                attn_flush_one()
    nc.compile()
    _nc_cache["nc"] = nc
    return nc


def _sub_order(h):
    return [h, 2 + h, 1 - h, 3 - h]


def _core_inputs(x, Wq, Wk, Wv, core):
    b, h = core // 2, core % 2
    sub = _sub_order(h)
    xkm = np.empty((C, T), dtype=np.float32)
    xb = np.asarray(x[b], dtype=np.float32)  # [T, C]
    for g in range(NG):
        for a, s in enumerate(sub):
            tlo = 128 * (4 * g + s)
            xkm[:, 512 * g + 128 * a: 512 * g + 128 * (a + 1)] = \
                xb[tlo:tlo + 128, :].T
    id2 = np.zeros((128, 64), dtype=np.float32)
    id2[:64] = np.eye(64, dtype=np.float32)
    id2[64:] = np.eye(64, dtype=np.float32)
    # diagonal-pair mask: cols [0,512) vs own tile s=h; [512,1024) vs s=2+h
    k = np.arange(128)[:, None]
    qcol = np.arange(512)[None, :]
    qoff = 128 * np.array(sub)[qcol // 128] + qcol % 128
    m0 = (qoff >= 128 * h + k)
    m1 = (qoff >= 128 * (2 + h) + k)
    mask = np.concatenate([m0, m1], axis=1).astype(np.float32)
    bf = ml_dtypes.bfloat16

    def warr(W):
        w = np.asarray(W, dtype=np.float32)
        return np.ascontiguousarray(
            w.reshape(NCH, 128, H).transpose(1, 0, 2).reshape(128, NCH * H)
            .astype(bf))

    return {
        "xk": np.ascontiguousarray(xkm.astype(bf)),
        "wq": warr(Wq),
        "wk": warr(Wk),
        "wv": warr(Wv),
        "ident2": id2.astype(bf),
        "maskd": np.ascontiguousarray(mask.astype(bf)),
    }


def kernel(x, Wq, Wk, Wv):
    x = np.asarray(x, dtype=np.float32)
    nc = build_module()
    in_maps = [_core_inputs(x, Wq, Wk, Wv, c) for c in range(N_CORES)]
    res = run_bass_kernel_spmd(nc, in_maps, core_ids=list(range(N_CORES)))
    out = np.empty((B, T, H), dtype=np.float32)
    inv = [np.argsort(_sub_order(h)) for h in range(2)]
    for b in range(B):
        pa = res.results[2 * b]["out"].astype(np.float64)   # [8, 65, 512]
        pb = res.results[2 * b + 1]["out"].astype(np.float64)
        # unpermute each core's query columns to global order, then combine
        pa = pa.reshape(NG, H + 1, 4, 128)[:, :, inv[0], :]
        pb = pb.reshape(NG, H + 1, 4, 128)[:, :, inv[1], :]
        num = pa[:, :H] + pb[:, :H]                  # [8, 64, 4, 128]
        den = pa[:, H] + pb[:, H]                    # [8, 4, 128]
        o = num / den[:, None, :, :]                 # [8, 64, 4, 128]
        out[b] = (o.transpose(0, 2, 3, 1)            # [8, 4, 128, 64]
                  .reshape(T, H).astype(np.float32))
    return out
